# revision 1
# baseline (speedup 1.0000x reference)
"""Distributed Trainium2 (Bass/Tile) kernel for the contrastive loss.

Strategy (8 NeuronCores, SPMD, row-sharded similarity matrix):
  Core c owns 512 of the 4096 rows of sim = reps @ reps^T (per l).
  The host rolls the column order by c*512 for each core so a single
  NEFF serves all cores: the self-match column for local row r is
  always column r, and the positive-pair column is always column
  r + 2048.  Each core:
    - loads all 4096 raw embedding rows (natural [row, d] layout),
    - L2-normalizes rows (fused square+row-sum on DVE; inv-norm via
      exp(-0.5*ln(ssq)) so every ACT op stays in one function-table set),
    - transposes normalized rows to [d, row] via PE matmuls vs identity,
    - computes its 512x4096 row-block of sim on PE (K=D=128 single shot),
    - exp(sim/T) + row-sum fused on ACT (activation accum_out),
    - extracts self/positive diagonal entries via masked
      tensor_tensor_reduce against identity on DVE,
    - combines into per-row weighted loss terms, DMAs out [128, 16].
  Host sums the 8 partial tensors -> scalar loss (the all-reduce).
"""

import numpy as np

TEMP = 0.2
L, B, K, D = 4, 64, 32, 128
N = B * K          # 2048
M = 2 * N          # 4096 rows of sim per l
NCORES = 8
R = M // NCORES    # 512 local rows per core
SEG = M // 128     # 32 row-tiles of 128 per l
INV_T = 1.0 / TEMP

_built = None


def _build():
    global _built
    if _built is not None:
        return _built
    from contextlib import ExitStack

    import concourse.tile as tile
    from concourse import bacc
    import concourse.mybir as mybir
    from concourse.masks import make_identity

    f32 = mybir.dt.float32
    AF = mybir.ActivationFunctionType
    OP = mybir.AluOpType
    AX = mybir.AxisListType

    # Pin every ACT op to the natural_log_exp_and_others table set (it covers
    # Copy/Exp/Identity/Ln/Square — everything we use), so bacc emits exactly
    # one LoadActFuncSet instead of thrashing ~2.7us loads between sets.
    # Other sets are EMPTIED (not removed) so act_func_set_id indices into
    # act_info.json stay valid.
    from concourse import hw_specs as _hw
    _tabs = dict(_hw.get_activation_tables("gen3"))
    _pinned = {
        name: (fns if name == "natural_log_exp_and_others" else frozenset())
        for name, fns in _tabs.items()
    }
    _hw.get_activation_tables.cache_clear()
    _orig = _hw.get_activation_tables.__wrapped__

    def _patched(arch):
        if arch == "gen3":
            return _pinned
        return _orig(arch)

    _hw.get_activation_tables = _patched
    import concourse.bacc as _baccmod
    if hasattr(_baccmod, "get_activation_tables"):
        _baccmod.get_activation_tables = _patched

    nc = bacc.Bacc(None, target_bir_lowering=False)
    emb = nc.dram_tensor("emb_nat", [128, L, SEG, D], f32, kind="ExternalInput")
    jvl = nc.dram_tensor("jv_local", [R], f32, kind="ExternalInput")
    out = nc.dram_tensor("out_wlp", [128, 4 * L], f32, kind="ExternalOutput")

    with ExitStack() as ctx:
        tc = ctx.enter_context(tile.TileContext(nc))
        singles = ctx.enter_context(tc.tile_pool(name="singles", bufs=1))
        natp = ctx.enter_context(tc.tile_pool(name="nat", bufs=8))
        xtp = ctx.enter_context(tc.tile_pool(name="xt", bufs=16))
        junkp = ctx.enter_context(tc.tile_pool(name="junk", bufs=4))
        statp = ctx.enter_context(tc.tile_pool(name="stat", bufs=2))
        expp = ctx.enter_context(tc.tile_pool(name="expo", bufs=4))
        tpp = ctx.enter_context(tc.tile_pool(name="tp", bufs=2, space="PSUM"))
        simp = ctx.enter_context(tc.tile_pool(name="sim", bufs=3, space="PSUM"))

        ident = singles.tile([128, 128], f32)
        make_identity(nc, ident[:])

        w = singles.tile([128, 4], f32)
        nc.sync.dma_start(out=w[:], in_=jvl.rearrange("(rb p) -> p rb", p=128))

        dsum = singles.tile([128, 4 * L, 4], f32)  # per (l,rb): 4 chunk sums
        selfb = singles.tile([128, 4 * L], f32)
        posb = singles.tile([128, 4 * L], f32)

        for l in range(L):
            # one contiguous DMA per l (128 x 16KB descriptors); issued from
            # the otherwise-idle GpSimd queue to keep SP free
            nat = natp.tile([128, SEG, D], f32)
            nc.gpsimd.dma_start(out=nat[:], in_=emb[:, l, :, :])

            ssq = statp.tile([128, SEG], f32)
            lnssq = statp.tile([128, SEG], f32)
            invn = statp.tile([128, SEG], f32)
            for g2 in range(4):
                for s8 in range(8):
                    s = g2 * 8 + s8
                    junk = junkp.tile([128, D], f32)
                    nc.vector.scalar_tensor_tensor(
                        out=junk[:], in0=nat[:, s, :], scalar=1.0,
                        in1=nat[:, s, :],
                        op0=OP.mult, op1=OP.mult, accum_out=ssq[:, s : s + 1])
                # inv_norm = exp(-0.5*ln(ssq)); Ln+Exp share one ACT table set
                sl = slice(g2 * 8, (g2 + 1) * 8)
                nc.scalar.activation(out=lnssq[:, sl], in_=ssq[:, sl], func=AF.Ln)
                nc.scalar.activation(
                    out=invn[:, sl], in_=lnssq[:, sl], func=AF.Exp, scale=-0.5)
                for s8 in range(8):
                    s = g2 * 8 + s8
                    nc.vector.tensor_scalar_mul(
                        nat[:, s, :], nat[:, s, :], invn[:, s : s + 1])

            # transpose normalized rows into [d, row] chunks of 512 columns
            xtc = []
            for g in range(8):
                ps = tpp.tile([128, 512], f32)
                for kk in range(4):
                    s = g * 4 + kk
                    nc.tensor.matmul(
                        ps[:, kk * 128 : (kk + 1) * 128], nat[:, s, :], ident[:],
                        start=True, stop=True)
                xc = xtp.tile([128, 512], f32)
                nc.vector.tensor_copy(xc[:], ps[:])
                xtc.append(xc)

            # the 512x4096 sim row-block for this l
            for rb in range(4):
                lr = l * 4 + rb
                lhsT = xtc[0][:, rb * 128 : (rb + 1) * 128]
                for t in range(4):
                    sim = simp.tile([128, 1024], f32)
                    for u in range(2):
                        fc = t * 2 + u
                        nc.tensor.matmul(
                            sim[:, u * 512 : (u + 1) * 512], lhsT,
                            xtc[fc][:],
                            start=True, stop=True)
                    eo = expp.tile([128, 1024], f32)
                    nc.scalar.activation(
                        out=eo[:], in_=sim[:], func=AF.Exp, scale=INV_T,
                        accum_out=dsum[:, lr, t : t + 1])
                    if t == 0 or t == 2:
                        # self-sim diagonal (t=0) / positive-pair diagonal (t=2)
                        buf = selfb if t == 0 else posb
                        junk = junkp.tile([128, 128], f32)
                        nc.vector.scalar_tensor_tensor(
                            out=junk[:], in0=sim[:, rb * 128 : rb * 128 + 128],
                            scalar=1.0, in1=ident[:],
                            op0=OP.mult, op1=OP.mult,
                            accum_out=buf[:, lr : lr + 1])

        # tail: per-row loss terms
        denom = singles.tile([128, 4 * L], f32)
        nc.vector.reduce_sum(out=denom[:], in_=dsum[:], axis=AX.X)
        selfexp = singles.tile([128, 4 * L], f32)
        nc.scalar.activation(out=selfexp[:], in_=selfb[:], func=AF.Exp, scale=INV_T)
        nc.vector.tensor_sub(denom[:], denom[:], selfexp[:])
        logd = singles.tile([128, 4 * L], f32)
        nc.scalar.activation(out=logd[:], in_=denom[:], func=AF.Ln)
        lp = singles.tile([128, 4 * L], f32)
        nc.vector.tensor_scalar_mul(lp[:], posb[:], -INV_T)
        nc.vector.tensor_add(lp[:], lp[:], logd[:])
        wlp = singles.tile([128, 4 * L], f32)
        for l in range(L):
            nc.vector.tensor_mul(
                wlp[:, l * 4 : (l + 1) * 4], lp[:, l * 4 : (l + 1) * 4], w[:])
        nc.sync.dma_start(out=out[:, :], in_=wlp[:])

    nc.finalize()
    _built = nc
    return nc


def _in_maps(emb_i, emb_j, joint_valid):
    emb_i = np.asarray(emb_i, dtype=np.float32)
    emb_j = np.asarray(emb_j, dtype=np.float32)
    jv = np.asarray(joint_valid, dtype=np.float32).reshape(-1)
    reps = np.concatenate(
        [emb_i.reshape(L, N, D), emb_j.reshape(L, N, D)], axis=1)  # [L, M, D]
    maps = []
    for c in range(NCORES):
        idx = (np.arange(M) + c * R) % M
        cols = reps[:, idx, :]  # rolled so local rows sit at columns 0..R-1
        nat = np.ascontiguousarray(
            cols.reshape(L, SEG, 128, D).transpose(2, 0, 1, 3))
        jvl = np.ascontiguousarray(jv[(np.arange(R) + c * R) % N])
        maps.append({"emb_nat": nat, "jv_local": jvl})
    return maps, jv


def _combine(results, jv):
    tot = 0.0
    for r in results:
        tot += float(r["out_wlp"].astype(np.float64).sum())
    return np.float32(tot / (2.0 * float(jv.sum())))


def kernel(emb_i, emb_j, joint_valid):
    from concourse.bass_utils import run_bass_kernel_spmd

    nc = _build()
    maps, jv = _in_maps(emb_i, emb_j, joint_valid)
    res = run_bass_kernel_spmd(nc, maps, core_ids=list(range(NCORES)))
    return _combine(res.results, jv)


def run_traced(inputs, trace_cores=None):
    """test.py helper: same run but with NTFF tracing enabled."""
    from concourse.bass_utils import run_bass_kernel_spmd

    nc = _build()
    maps, jv = _in_maps(**inputs)
    res = run_bass_kernel_spmd(
        nc, maps, core_ids=list(range(NCORES)), trace=True,
        trace_cores=trace_cores if trace_cores is not None else list(range(NCORES)))
    res.loss = _combine(res.results, jv)
    return res



# revision 6
# speedup vs baseline: 1.2263x; 1.2263x over previous
"""Distributed Trainium2 (Bass/Tile) kernel for the contrastive loss.

Strategy (8 NeuronCores, SPMD, row-sharded similarity matrix):
  Core c owns 512 of the 4096 rows of sim = reps @ reps^T (per l).
  The host rolls the column order by c*512 for each core so a single
  NEFF serves all cores: the self-match column for local row r is
  always column r, and the positive-pair column is always column
  r + 2048.  Each core:
    - loads all 4096 raw embedding rows (natural [row, d] layout),
    - L2-normalizes rows (fused square+row-sum on DVE; inv-norm via
      exp(-0.5*ln(ssq)) so every ACT op stays in one function-table set),
    - transposes normalized rows to [d, row] via PE matmuls vs identity,
    - computes its 512x4096 row-block of sim on PE (K=D=128 single shot),
    - exp(sim/T) + row-sum fused on ACT (activation accum_out),
    - extracts self/positive diagonal entries via masked
      tensor_tensor_reduce against identity on DVE,
    - combines into per-row weighted loss terms, DMAs out [128, 16].
  Host sums the 8 partial tensors -> scalar loss (the all-reduce).
"""

import numpy as np

TEMP = 0.2
L, B, K, D = 4, 64, 32, 128
N = B * K          # 2048
M = 2 * N          # 4096 rows of sim per l
NCORES = 8
R = M // NCORES    # 512 local rows per core
SEG = M // 128     # 32 row-tiles of 128 per l
INV_T = 1.0 / TEMP

_built = None


def _build():
    global _built
    if _built is not None:
        return _built
    from contextlib import ExitStack

    import concourse.tile as tile
    from concourse import bacc
    import concourse.mybir as mybir
    from concourse.masks import make_identity

    f32 = mybir.dt.float32
    AF = mybir.ActivationFunctionType
    OP = mybir.AluOpType
    AX = mybir.AxisListType

    # Pin every ACT op to the natural_log_exp_and_others table set (it covers
    # Copy/Exp/Identity/Ln/Square — everything we use), so bacc emits exactly
    # one LoadActFuncSet instead of thrashing ~2.7us loads between sets.
    # Other sets are EMPTIED (not removed) so act_func_set_id indices into
    # act_info.json stay valid.
    from concourse import hw_specs as _hw
    _tabs = dict(_hw.get_activation_tables("gen3"))
    _pinned = {
        name: (fns if name == "natural_log_exp_and_others" else frozenset())
        for name, fns in _tabs.items()
    }
    _hw.get_activation_tables.cache_clear()
    _orig = _hw.get_activation_tables.__wrapped__

    def _patched(arch):
        if arch == "gen3":
            return _pinned
        return _orig(arch)

    _hw.get_activation_tables = _patched
    import concourse.bacc as _baccmod
    if hasattr(_baccmod, "get_activation_tables"):
        _baccmod.get_activation_tables = _patched

    nc = bacc.Bacc(None, target_bir_lowering=False)
    emb = nc.dram_tensor("emb_nat", [128, L, SEG, D], f32, kind="ExternalInput")
    jvl = nc.dram_tensor("jv_local", [R], f32, kind="ExternalInput")
    out = nc.dram_tensor("out_wlp", [128, 4 * L], f32, kind="ExternalOutput")

    with ExitStack() as ctx:
        tc = ctx.enter_context(tile.TileContext(nc))
        singles = ctx.enter_context(tc.tile_pool(name="singles", bufs=1))
        natp = ctx.enter_context(tc.tile_pool(name="nat", bufs=8))
        xtp = ctx.enter_context(tc.tile_pool(name="xt", bufs=16))
        junkp = ctx.enter_context(tc.tile_pool(name="junk", bufs=4))
        statp = ctx.enter_context(tc.tile_pool(name="stat", bufs=2))
        expp = ctx.enter_context(tc.tile_pool(name="expo", bufs=4))
        tpp = ctx.enter_context(tc.tile_pool(name="tp", bufs=2, space="PSUM"))
        simp = ctx.enter_context(tc.tile_pool(name="sim", bufs=3, space="PSUM"))

        ident = singles.tile([128, 128], f32)
        make_identity(nc, ident[:])

        w = singles.tile([128, 4], f32)
        nc.sync.dma_start(out=w[:], in_=jvl.rearrange("(rb p) -> p rb", p=128))

        dsum = singles.tile([128, 4 * L, 4], f32)  # per (l,rb): 4 chunk sums
        selfb = singles.tile([128, 4 * L], f32)
        posb = singles.tile([128, 4 * L], f32)

        for l in range(L):
            # one contiguous DMA per l (128 x 16KB descriptors); issued from
            # the otherwise-idle GpSimd queue to keep SP free
            nat = natp.tile([128, SEG, D], f32)
            nc.gpsimd.dma_start(out=nat[:], in_=emb[:, l, :, :])

            ssq = statp.tile([128, SEG], f32)
            lnssq = statp.tile([128, SEG], f32)
            invn = statp.tile([128, SEG], f32)
            for g2 in range(4):
                for s8 in range(8):
                    s = g2 * 8 + s8
                    junk = junkp.tile([128, D], f32)
                    nc.vector.scalar_tensor_tensor(
                        out=junk[:], in0=nat[:, s, :], scalar=1.0,
                        in1=nat[:, s, :],
                        op0=OP.mult, op1=OP.mult, accum_out=ssq[:, s : s + 1])
                # inv_norm = exp(-0.5*ln(ssq)); Ln+Exp share one ACT table set
                sl = slice(g2 * 8, (g2 + 1) * 8)
                nc.scalar.activation(out=lnssq[:, sl], in_=ssq[:, sl], func=AF.Ln)
                nc.scalar.activation(
                    out=invn[:, sl], in_=lnssq[:, sl], func=AF.Exp, scale=-0.5)
                for s8 in range(8):
                    s = g2 * 8 + s8
                    nc.vector.tensor_scalar_mul(
                        nat[:, s, :], nat[:, s, :], invn[:, s : s + 1])

            # transpose normalized rows into [d, row] chunks of 512 columns
            xtc = []
            for g in range(8):
                ps = tpp.tile([128, 512], f32)
                for kk in range(4):
                    s = g * 4 + kk
                    nc.tensor.matmul(
                        ps[:, kk * 128 : (kk + 1) * 128], nat[:, s, :], ident[:],
                        start=True, stop=True)
                xc = xtp.tile([128, 512], mybir.dt.float32r)
                nc.vector.tensor_copy(xc[:], ps[:])
                xtc.append(xc)

            # the 512x4096 sim row-block for this l
            for rb in range(4):
                lr = l * 4 + rb
                lhsT = xtc[0][:, rb * 128 : (rb + 1) * 128]
                for t in range(4):
                    sim = simp.tile([128, 1024], f32)
                    for u in range(2):
                        fc = t * 2 + u
                        nc.tensor.matmul(
                            sim[:, u * 512 : (u + 1) * 512],
                            lhsT, xtc[fc][:],
                            start=True, stop=True)
                    eo = expp.tile([128, 1024], f32)
                    nc.scalar.activation(
                        out=eo[:], in_=sim[:], func=AF.Exp, scale=INV_T,
                        accum_out=dsum[:, lr, t : t + 1])
                    if t == 0 or t == 2:
                        # self-sim diagonal (t=0) / positive-pair diagonal (t=2)
                        buf = selfb if t == 0 else posb
                        junk = junkp.tile([128, 128], f32)
                        nc.vector.scalar_tensor_tensor(
                            out=junk[:], in0=sim[:, rb * 128 : rb * 128 + 128],
                            scalar=1.0, in1=ident[:],
                            op0=OP.mult, op1=OP.mult,
                            accum_out=buf[:, lr : lr + 1])

        # tail: per-row loss terms
        denom = singles.tile([128, 4 * L], f32)
        nc.vector.reduce_sum(out=denom[:], in_=dsum[:], axis=AX.X)
        selfexp = singles.tile([128, 4 * L], f32)
        nc.scalar.activation(out=selfexp[:], in_=selfb[:], func=AF.Exp, scale=INV_T)
        nc.vector.tensor_sub(denom[:], denom[:], selfexp[:])
        logd = singles.tile([128, 4 * L], f32)
        nc.scalar.activation(out=logd[:], in_=denom[:], func=AF.Ln)
        lp = singles.tile([128, 4 * L], f32)
        nc.vector.tensor_scalar_mul(lp[:], posb[:], -INV_T)
        nc.vector.tensor_add(lp[:], lp[:], logd[:])
        wlp = singles.tile([128, 4 * L], f32)
        for l in range(L):
            nc.vector.tensor_mul(
                wlp[:, l * 4 : (l + 1) * 4], lp[:, l * 4 : (l + 1) * 4], w[:])
        nc.sync.dma_start(out=out[:, :], in_=wlp[:])

    nc.finalize()
    _built = nc
    return nc


def _in_maps(emb_i, emb_j, joint_valid):
    emb_i = np.asarray(emb_i, dtype=np.float32)
    emb_j = np.asarray(emb_j, dtype=np.float32)
    jv = np.asarray(joint_valid, dtype=np.float32).reshape(-1)
    reps = np.concatenate(
        [emb_i.reshape(L, N, D), emb_j.reshape(L, N, D)], axis=1)  # [L, M, D]
    maps = []
    for c in range(NCORES):
        idx = (np.arange(M) + c * R) % M
        cols = reps[:, idx, :]  # rolled so local rows sit at columns 0..R-1
        nat = np.ascontiguousarray(
            cols.reshape(L, SEG, 128, D).transpose(2, 0, 1, 3))
        jvl = np.ascontiguousarray(jv[(np.arange(R) + c * R) % N])
        maps.append({"emb_nat": nat, "jv_local": jvl})
    return maps, jv


def _combine(results, jv):
    tot = 0.0
    for r in results:
        tot += float(r["out_wlp"].astype(np.float64).sum())
    return np.float32(tot / (2.0 * float(jv.sum())))


def kernel(emb_i, emb_j, joint_valid):
    from concourse.bass_utils import run_bass_kernel_spmd

    nc = _build()
    maps, jv = _in_maps(emb_i, emb_j, joint_valid)
    res = run_bass_kernel_spmd(nc, maps, core_ids=list(range(NCORES)))
    return _combine(res.results, jv)


def run_traced(inputs, trace_cores=None):
    """test.py helper: same run but with NTFF tracing enabled."""
    from concourse.bass_utils import run_bass_kernel_spmd

    nc = _build()
    maps, jv = _in_maps(**inputs)
    res = run_bass_kernel_spmd(
        nc, maps, core_ids=list(range(NCORES)), trace=True,
        trace_cores=trace_cores if trace_cores is not None else list(range(NCORES)))
    res.loss = _combine(res.results, jv)
    return res



# revision 9
# speedup vs baseline: 2.4274x; 1.9794x over previous
"""Distributed Trainium2 (Bass/Tile) kernel for the contrastive loss.

Ring-partitioned symmetric-similarity scheme (8 NeuronCores, SPMD):

  Global per l: 4096 rows in 32 chunks of 128.  sim = Z Z^T is symmetric,
  so each unordered chunk pair is computed ONCE: chunk i covers column
  chunks {i..i+15} (ring-forward), and the distance-16 block is computed
  by both endpoints with its exp HALVED (bias ln 1/2).  Core c owns row
  chunks {4c..4c+3}; it therefore only needs Z chunks {4c..4c+19} (a
  20-chunk window, rolled so the window is local chunks 0..19).

  Per core:
    - load raw window rows in natural layout [128, l, 20, 128] (fp32),
    - ssq via fused square+row-sum on DVE; invn = exp(-0.5 ln ssq) (ACT),
    - scale rows by invn -> bf16 zb (GpSimd tensor_scalar),
    - transpose zb via the XBAR DMA-transpose -> xt [d, chunk, row] bf16,
    - row-chunk i: 4x 512-col bf16 matmuls -> PSUM [128, 2048];
      ACT exp (scale 1/T) -> E bf16 SBUF + per-row accum (denominator
      row part);  per covered column chunk a 1-col ones-matmul on PE
      computes the column sums (the denominator part owed to OTHER
      row chunks); distance-16 block separately with bias ln(1/2),
    - positive pairs are exactly the distance-16 diagonals:
      pos = zb[:,i,:] . zb[:,i+16,:] row-dots on DVE.
  Outputs per core: row accums, d16 row sums, column-sum partials and
  pos dots.  The host does the tiny cross-core assembly: denominators =
  row part + mapped column partials - e^5 (self term), then
  loss = sum w (-pos/T + log denom) / (2 sum w).
"""

import numpy as np

TEMP = 0.2
INV_T = 1.0 / TEMP
L, B, K, D = 4, 64, 32, 128
N = B * K            # 2048
M = 2 * N            # 4096 rows per l
NCH = 32             # global 128-row chunks per l
NCORES = 8
RC = 4               # row chunks owned per core
W = 20               # chunk window per core (rc spans + d16 partners)
SPAN = 16            # forward span chunks (excl. the halved d16 block)

_built = None


def _build():
    global _built
    if _built is not None:
        return _built
    from contextlib import ExitStack

    import concourse.tile as tile
    from concourse import bacc
    import concourse.mybir as mybir

    f32 = mybir.dt.float32
    bf16 = mybir.dt.bfloat16
    AF = mybir.ActivationFunctionType
    OP = mybir.AluOpType
    AX = mybir.AxisListType

    # Pin every ACT op to the natural_log_exp_and_others table set (covers
    # Copy/Exp/Identity/Ln) so bacc emits exactly one LoadActFuncSet.
    from concourse import hw_specs as _hw
    _tabs = dict(_hw.get_activation_tables("gen3"))
    _pinned = {
        name: (fns if name == "natural_log_exp_and_others" else frozenset())
        for name, fns in _tabs.items()
    }
    _hw.get_activation_tables.cache_clear()
    _orig = _hw.get_activation_tables.__wrapped__

    def _patched(arch):
        if arch == "gen3":
            return _pinned
        return _orig(arch)

    _hw.get_activation_tables = _patched
    import concourse.bacc as _baccmod
    if hasattr(_baccmod, "get_activation_tables"):
        _baccmod.get_activation_tables = _patched

    nc = bacc.Bacc(None, target_bir_lowering=False)
    emb = nc.dram_tensor("emb_nat", [128, L, W, D], f32, kind="ExternalInput")
    o_dsum = nc.dram_tensor("o_dsum", [128, L * RC], f32, kind="ExternalOutput")
    o_d16r = nc.dram_tensor("o_d16r", [128, L * RC], f32, kind="ExternalOutput")
    o_pos = nc.dram_tensor("o_pos", [128, L * RC], f32, kind="ExternalOutput")
    o_cacc = nc.dram_tensor("o_cacc", [128, L, W], f32, kind="ExternalOutput")

    with ExitStack() as ctx:
        tc = ctx.enter_context(tile.TileContext(nc))
        singles = ctx.enter_context(tc.tile_pool(name="singles", bufs=1))
        natp = ctx.enter_context(tc.tile_pool(name="nat", bufs=2))
        zbp = ctx.enter_context(tc.tile_pool(name="zb", bufs=2))
        xtp = ctx.enter_context(tc.tile_pool(name="xt", bufs=2))
        statp = ctx.enter_context(tc.tile_pool(name="stat", bufs=2))
        ep = ctx.enter_context(tc.tile_pool(name="eo", bufs=3))
        e16p = ctx.enter_context(tc.tile_pool(name="e16", bufs=2))
        simp = ctx.enter_context(tc.tile_pool(name="sim", bufs=2, space="PSUM"))

        ones_bf = singles.tile([128, 1], bf16)
        nc.vector.memset(ones_bf[:], 1.0)
        ln_half = singles.tile([128, 1], f32)
        nc.vector.memset(ln_half[:], float(np.log(0.5)))
        dsum = singles.tile([128, L * RC], f32)
        d16r = singles.tile([128, L * RC], f32)
        pos = singles.tile([128, L * RC], f32)
        cacc = singles.tile([128, L, W], f32)
        nc.vector.memset(cacc[:], 0.0)
        junk = singles.tile([128, D], f32)
        junkb = singles.tile([128, D], bf16)

        nats, zbs, xts = {}, {}, {}

        def prep(l):
            nat = natp.tile([128, W, D], f32)
            nc.sync.dma_start(out=nat[:], in_=emb[:, l, :, :])
            ssq = statp.tile([128, W], f32)
            lnssq = statp.tile([128, W], f32)
            invn = statp.tile([128, W], f32)
            for s in range(W):
                nc.vector.scalar_tensor_tensor(
                    out=junk[:], in0=nat[:, s, :], scalar=1.0, in1=nat[:, s, :],
                    op0=OP.mult, op1=OP.mult, accum_out=ssq[:, s : s + 1])
            nc.scalar.activation(out=lnssq[:], in_=ssq[:], func=AF.Ln)
            nc.scalar.activation(out=invn[:], in_=lnssq[:], func=AF.Exp,
                                 scale=-0.5)
            zb = zbp.tile([128, W, D], bf16)
            for s in range(W):
                nc.gpsimd.tensor_scalar_mul(
                    zb[:, s, :], nat[:, s, :], invn[:, s : s + 1])
            xt = xtp.tile([128, W, 128], bf16)
            nc.sync.dma_start_transpose(
                out=xt[:], in_=zb[:].rearrange("p s d -> p (s d)"))
            nats[l], zbs[l], xts[l] = nat, zb, xt

        def sims(l):
            xt = xts[l]
            zb = zbs[l]
            xtf = xt[:].rearrange("p s r -> p (s r)")
            sts, es = [], []
            # all span matmuls + exps first (PE slots ahead of colsums)
            for i in range(RC):
                st = simp.tile([128, SPAN * 128], f32, tag="st")
                for q in range(4):
                    c0 = i * 128 + q * 512
                    nc.tensor.matmul(
                        st[:, q * 512 : (q + 1) * 512], xt[:, i, :],
                        xtf[:, c0 : c0 + 512], start=True, stop=True)
                e = ep.tile([128, SPAN * 128], bf16)
                nc.scalar.activation(
                    out=e[:], in_=st[:], func=AF.Exp, scale=INV_T,
                    accum_out=dsum[:, l * RC + i : l * RC + i + 1])
                # column sums of the non-diagonal blocks (chunks i+1..i+15)
                for k in range(1, SPAN):
                    nc.tensor.matmul(
                        st[:, k - 1 : k], e[:, k * 128 : (k + 1) * 128],
                        ones_bf[:], start=True, stop=True)
                nc.vector.tensor_tensor(
                    out=cacc[:, l, i + 1 : i + SPAN],
                    in0=cacc[:, l, i + 1 : i + SPAN],
                    in1=st[:, 0 : SPAN - 1], op=OP.add)
                sts.append(st)
                es.append(e)
            # distance-16 blocks, exp halved via bias ln(1/2)
            st2 = simp.tile([128, SPAN * 128], f32, tag="st")
            for i in range(RC):
                nc.tensor.matmul(
                    st2[:, i * 128 : (i + 1) * 128], xt[:, i, :],
                    xt[:, i + SPAN, :], start=True, stop=True)
            e16 = e16p.tile([128, RC, 128], bf16)
            nc.scalar.activation(
                out=e16[:].rearrange("p a b -> p (a b)"), in_=st2[:, 0:512],
                func=AF.Exp, scale=INV_T, bias=ln_half[:])
            nc.vector.reduce_sum(
                out=d16r[:, l * RC : (l + 1) * RC], in_=e16[:], axis=AX.X)
            for i in range(RC):
                nc.tensor.matmul(
                    st2[:, 512 + i : 513 + i], e16[:, i, :], ones_bf[:],
                    start=True, stop=True)
            nc.vector.tensor_tensor(
                out=cacc[:, l, SPAN : SPAN + RC],
                in0=cacc[:, l, SPAN : SPAN + RC],
                in1=st2[:, 512 : 512 + RC], op=OP.add)
            # positives = distance-16 diagonals = row-dots of zb chunk pairs
            for i in range(RC):
                nc.vector.scalar_tensor_tensor(
                    out=junkb[:], in0=zb[:, i, :], scalar=1.0,
                    in1=zb[:, i + SPAN, :], op0=OP.mult, op1=OP.mult,
                    accum_out=pos[:, l * RC + i : l * RC + i + 1])

        prep(0)
        for l in range(L):
            if l + 1 < L:
                prep(l + 1)
            sims(l)

        nc.sync.dma_start(out=o_dsum[:, :], in_=dsum[:])
        nc.sync.dma_start(out=o_d16r[:, :], in_=d16r[:])
        nc.sync.dma_start(out=o_pos[:, :], in_=pos[:])
        nc.sync.dma_start(out=o_cacc[:, :, :], in_=cacc[:])

    nc.finalize()
    _built = nc
    return nc


def _in_maps(emb_i, emb_j, joint_valid):
    emb_i = np.asarray(emb_i, dtype=np.float32)
    emb_j = np.asarray(emb_j, dtype=np.float32)
    jv = np.asarray(joint_valid, dtype=np.float32).reshape(-1)
    reps = np.concatenate(
        [emb_i.reshape(L, N, D), emb_j.reshape(L, N, D)], axis=1)  # [L, M, D]
    repsc = reps.reshape(L, NCH, 128, D)
    maps = []
    for c in range(NCORES):
        sel = (RC * c + np.arange(W)) % NCH
        win = repsc[:, sel]                       # [L, W, 128, D]
        nat = np.ascontiguousarray(win.transpose(2, 0, 1, 3))  # [128, L, W, D]
        maps.append({"emb_nat": nat})
    return maps, jv


def _combine(results, jv):
    E5 = float(np.exp(INV_T))  # self-similarity exp(1/T)
    denom = np.zeros((L, NCH, 128), dtype=np.float64)
    posg = np.zeros((L, NCH, 128), dtype=np.float64)
    for c, r in enumerate(results):
        dsum = r["o_dsum"].astype(np.float64)   # [128, L*RC]
        d16r = r["o_d16r"].astype(np.float64)
        pos = r["o_pos"].astype(np.float64)
        cacc = r["o_cacc"].astype(np.float64)   # [128, L, W]
        for l in range(L):
            for i in range(RC):
                g = (RC * c + i) % NCH
                denom[l, g] += dsum[:, l * RC + i] + d16r[:, l * RC + i]
                posg[l, g] = pos[:, l * RC + i]
            for j in range(1, W):
                g = (RC * c + j) % NCH
                denom[l, g] += cacc[:, l, j]
    denom -= E5
    w = jv.astype(np.float64)                   # [N]
    wrow = np.concatenate([w, w]).reshape(NCH, 128)  # weight per global row
    lp = -posg * INV_T + np.log(denom)          # [L, NCH, 128]
    loss = (lp * wrow[None]).sum() / (2.0 * w.sum())
    return np.float32(loss)


def kernel(emb_i, emb_j, joint_valid):
    from concourse.bass_utils import run_bass_kernel_spmd

    nc = _build()
    maps, jv = _in_maps(emb_i, emb_j, joint_valid)
    res = run_bass_kernel_spmd(nc, maps, core_ids=list(range(NCORES)))
    return _combine(res.results, jv)


def run_traced(inputs, trace_cores=None):
    """test.py helper: same run but with NTFF tracing enabled."""
    from concourse.bass_utils import run_bass_kernel_spmd

    nc = _build()
    maps, jv = _in_maps(**inputs)
    res = run_bass_kernel_spmd(
        nc, maps, core_ids=list(range(NCORES)), trace=True,
        trace_cores=trace_cores if trace_cores is not None else list(range(NCORES)))
    res.loss = _combine(res.results, jv)
    return res
